# revision 19
# baseline (speedup 1.0000x reference)
"""Trainium2 Bass kernel for GQA attention block (nn_Attention_20272245637793).

Reference computation (B=2, S=2048, H=2048, 16 q heads / 8 kv heads, D=128):
    q = hs @ Wq.T ; k = hs @ Wk.T ; v = hs @ Wv.T
    rope(q), rope(k); causal softmax(q k^T / sqrt(D)) @ v ; out @ Wo.T

Sharding (8 cores): core i = (b, g) with b = i // 4 (data-parallel over
batch), g = i % 4 (tensor-parallel over kv-head groups; kv heads {2g, 2g+1},
q heads {4g..4g+3}).  Each core computes 1/8 of every GEMM and a partial
o_proj over its 512 head-dims; the host sums the 4 partials per batch
(cheap, off-device) instead of an on-device all-reduce.

Per-core dataflow (matmul operands fp16, PSUM accumulation fp32):
  phase 1: QK^T projections produce q^T/k^T in [d_head(part) x S(free)]
           layout directly (weights stationary, hs^T moving); RoPE applied
           on the PSUM->SBUF path with 4 DVE ops per tile using
           host-precomputed cos / (+/-)sin tables.  V is computed
           NON-transposed ([S x d]) by using hs^T slices as the stationary
           operand, and gets a ones-column appended (denominator trick).
  phase 2: per q head: scores^T tiles = K^T-chunk (stationary) @ q^T
           (moving) -> PSUM [k_pos(part) x q(free)] in 1024-wide 2-bank
           tiles (2 matmuls fill, ONE wide exp drains via ScalarE with
           scale=1/sqrt(D) fused); causal handled by skipping fully-masked
           tiles + one 0/1 mask multiply on diagonal tiles.  PV: exp'd
           score tiles are the stationary operand against V' (with ones
           column) -> PSUM [q(part) x 129]; col 128 is the softmax
           denominator; normalize with reciprocal + per-partition scalar
           multiply into an SBUF staging tile; the [q x d] -> [d x q]
           transpose for o_proj runs OFF the PE as a DMA XBAR transpose
           on the idle sync ring.
  phase 3: o_proj partial out^T[h, s] = Wo-slice^T (stationary) @ attn^T
           (moving), mt-outer so each 128-row strip is stored with ONE
           wide fp16 DMA; host sums/transposes partials in fp32.

DMA layout: all input tensors are shaped so per-partition lines are
4-16 KiB contiguous (whole-tensor or half-tensor DMAs) -- descriptor
count is what gates the kernel lead-in, not bandwidth.

Built on bacc.Bacc (not raw bass.Bass): TRN2 instructions can carry at most
ONE semaphore wait; Bacc.compile() legalizes multi-wait instructions via
move_matmul_waits_to_ldweights + generate_event_semaphores.
"""

import sys

sys.path.insert(0, "/opt/trn_rl_repo")

import numpy as np
from contextlib import ExitStack

B = 2
S = 2048
H = 2048
D = 128
NQ = 4          # q heads per core
NKVL = 2        # kv heads per core
HC = H // 128   # 16 h-chunks (contraction)
NB = 8          # hs^T column blocks of 256 for projections
BW = S // NB    # 256
ST = S // 128   # 16 s-tiles / k-chunks / q-tiles
SCALE = 1.0 / np.sqrt(D)

# stripe c of the exp'd transposed scores covers q in [128c, S); offsets of
# the stripes packed into one [128, sum] sbuf tile
STRIPE_LEN = [S - 128 * c for c in range(ST)]
STRIPE_OFF = np.concatenate([[0], np.cumsum(STRIPE_LEN)]).tolist()
PT_TOTAL = STRIPE_OFF[-1]  # 17408

# dtype for matmul operands (PSUM accumulation is always fp32).  fp32 runs
# every matmul as a hi/lo double pass on the PE; float16 is single-pass
# (2x) with ~1e-3 accuracy, and halves DMA/SBUF for those tensors.
MM_DT = "float16"

_CACHE = {}


def _build_program():
    import concourse.tile as tile
    from concourse import bacc, mybir

    f32 = mybir.dt.float32
    fmm = getattr(mybir.dt, MM_DT)
    nc = bacc.Bacc()

    hsT_d = nc.declare_dram_parameter("hsT", [NB, 128, HC, BW], fmm, isOutput=False)
    wq_d = nc.declare_dram_parameter("wq", [128, HC, 128 * NQ], fmm, isOutput=False)
    wk_d = nc.declare_dram_parameter("wk", [128, HC, 128 * NKVL], fmm, isOutput=False)
    wv_d = nc.declare_dram_parameter("wv", [128, HC, 128 * NKVL], fmm, isOutput=False)
    wo_d = nc.declare_dram_parameter("wo", [128, NQ, H], fmm, isOutput=False)
    cos_d = nc.declare_dram_parameter("cosf", [128, S], fmm, isOutput=False)
    sin_d = nc.declare_dram_parameter("sins", [128, S], fmm, isOutput=False)
    mask_d = nc.declare_dram_parameter("mask", [128, 128], fmm, isOutput=False)
    outT_d = nc.declare_dram_parameter("outT", [H, S], fmm, isOutput=True)
    outT2_d = nc.declare_dram_parameter("outT2", [H, S], fmm, isOutput=True)

    with tile.TileContext(nc) as tc, ExitStack() as top:
        # tiles that live across phases
        glob = top.enter_context(tc.tile_pool(name="glob", bufs=1))
        qrot = glob.tile([128, NQ, S], fmm)      # q^T, rope'd, per head
        krot = glob.tile([128, NKVL, S], fmm)    # k^T, rope'd, per kv head
        vaug = glob.tile([128, NKVL, ST, 132], fmm)  # v chunks + ones col @128
        mask_sb = glob.tile([128, 128], fmm)

        nc.gpsimd.dma_start(out=mask_sb, in_=mask_d[:, :])
        nc.vector.memset(vaug[:, :, :, 128:129], 1.0)

        # HAM warm-up: ~10 throwaway matmuls on a zeroed scratch tile keep
        # the PE busy across the 3.4us activity window while the first
        # input DMAs land, so real matmuls start at the full clock.
        with ExitStack() as wctx:
            wpool = wctx.enter_context(tc.tile_pool(name="warm", bufs=1))
            wps_pool = wctx.enter_context(
                tc.tile_pool(name="warmps", bufs=1, space="PSUM")
            )
            warm = wpool.tile([128, 512], fmm)
            nc.vector.memset(warm, 0.0)
            wps = wps_pool.tile([128, 512], f32)
            for _ in range(10):
                nc.tensor.matmul(wps, warm[:, 0:128], warm, start=True, stop=True)

        # ---------------- phase 1: projections + rope ----------------
        with ExitStack() as ph1:
            consts = ph1.enter_context(tc.tile_pool(name="p1const", bufs=1))
            hsp = ph1.enter_context(tc.tile_pool(name="p1hs", bufs=3))
            ropep = ph1.enter_context(tc.tile_pool(name="p1rope", bufs=3))
            qk_ps = ph1.enter_context(tc.tile_pool(name="p1qkps", bufs=3, space="PSUM"))
            v_ps = ph1.enter_context(tc.tile_pool(name="p1vps", bufs=2, space="PSUM"))

            def hs_load(nb, splits=(4, 8, 12, 16)):
                # few DMAs per 1 MiB block with 2+ KiB contiguous
                # per-partition lines (vs 512 B per-chunk lines --
                # descriptor count gates the kernel lead-in, not bandwidth)
                t = hsp.tile([128, HC, BW], fmm, name=f"hs_{nb}", tag="hs")
                lo = 0
                for hi in splits:
                    nc.sync.dma_start(
                        out=t[:, lo:hi, :], in_=hsT_d[nb, :, lo:hi, :]
                    )
                    lo = hi
                return t

            wq_sb = consts.tile([128, HC, 128 * NQ], fmm)
            wk_sb = consts.tile([128, HC, 128 * NKVL], fmm)
            wv_sb = consts.tile([128, HC, 128 * NKVL], fmm)
            cos_sb = consts.tile([128, S], fmm)
            sin_sb = consts.tile([128, S], fmm)
            # scalar ring, in consumption order: a small first wq slice so
            # the very first matmuls can start, the rest of wq, then trig
            # in halves (rope of nb=0 only needs the first half).
            for lo, hi in ((0, 1), (1, 4), (4, 10), (10, 16)):
                nc.scalar.dma_start(
                    out=wq_sb[:, lo:hi, :], in_=wq_d[:, lo:hi, :]
                )
                if hi == 4:
                    nc.scalar.dma_start(
                        out=cos_sb[:, 0:1024], in_=cos_d[:, 0:1024]
                    )
                    nc.scalar.dma_start(
                        out=sin_sb[:, 0:1024], in_=sin_d[:, 0:1024]
                    )
            nc.scalar.dma_start(out=cos_sb[:, 1024:S], in_=cos_d[:, 1024:S])
            nc.scalar.dma_start(out=sin_sb[:, 1024:S], in_=sin_d[:, 1024:S])

            hs_next = hs_load(0, splits=(2, 4, 8, 12, 16))
            nc.sync.dma_start(out=wk_sb, in_=wk_d[:, :, :])
            nc.sync.dma_start(out=wv_sb, in_=wv_d[:, :, :])

            for nb in range(NB):
                n0 = nb * BW
                hs_t = hs_next
                if nb + 1 < NB:
                    hs_next = hs_load(nb + 1)

                # q/k projections (transposed out) + rope
                for mt in range(NQ + NKVL):
                    ps = qk_ps.tile([128, BW], f32)
                    if mt < NQ:
                        w_sb, mo = wq_sb, mt
                    else:
                        w_sb, mo = wk_sb, mt - NQ
                    for c in range(HC):
                        nc.tensor.matmul(
                            ps,
                            w_sb[:, c, 128 * mo : 128 * mo + 128],
                            hs_t[:, c, :],
                            start=(c == 0),
                            stop=(c == HC - 1),
                        )
                    if mt < NQ:
                        dest = qrot[:, mt, n0 : n0 + BW]
                    else:
                        dest = krot[:, mt - NQ, n0 : n0 + BW]
                    # rope: dest = ps * cos + swap_halves(ps) * (+/-)sin
                    t_t = ropep.tile([128, BW], f32, tag="ropet")
                    u_t = ropep.tile([128, BW], f32, tag="ropeu")
                    nc.vector.tensor_mul(t_t, ps, cos_sb[:, n0 : n0 + BW])
                    nc.vector.tensor_mul(
                        u_t[0:64, :], ps[64:128, :], sin_sb[0:64, n0 : n0 + BW]
                    )
                    nc.vector.tensor_mul(
                        u_t[64:128, :], ps[0:64, :], sin_sb[64:128, n0 : n0 + BW]
                    )
                    nc.vector.tensor_add(dest, t_t, u_t)

                # v projection (NOT transposed): out[s, d_local]
                for st2 in range(BW // 128):
                    st = (BW // 128) * nb + st2
                    ps = v_ps.tile([128, 128 * NKVL], f32)
                    for c in range(HC):
                        nc.tensor.matmul(
                            ps,
                            hs_t[:, c, 128 * st2 : 128 * st2 + 128],
                            wv_sb[:, c, :],
                            start=(c == 0),
                            stop=(c == HC - 1),
                        )
                    for kv in range(NKVL):
                        # ScalarE copy: ACT is idle during phase 1, DVE is not
                        nc.scalar.copy(
                            vaug[:, kv, st, 0:128], ps[:, 128 * kv : 128 * kv + 128]
                        )

        # ---------------- phases 2+3 ----------------
        late = top.enter_context(tc.tile_pool(name="late", bufs=1))
        attnT = late.tile([128, NQ, S], fmm)     # attention out, transposed
        wo_sb = late.tile([128, NQ, H], fmm)
        nc.gpsimd.dma_start(out=wo_sb, in_=wo_d[:, :, :])

        # ---------------- phase 2: attention ----------------
        # While heads 2/3 run their attention (which is paced by the
        # ScalarE exp stream, leaving PE bubbles), the o_proj partial for
        # heads 0/1 is interleaved into the PE queue: 64 (mt, ns) groups
        # of 2 accumulating matmuls each, drained to a second DRAM output
        # (outT2) that the host sums with the heads-2/3 partial.
        with ExitStack() as ph2:
            ptp = ph2.enter_context(tc.tile_pool(name="p2pt", bufs=2))
            s_ps = ph2.enter_context(tc.tile_pool(name="p2sps", bufs=2, space="PSUM"))
            pv_ps = ph2.enter_context(tc.tile_pool(name="p2pvps", bufs=2, space="PSUM"))
            o01_ps = ph2.enter_context(
                tc.tile_pool(name="p2o01ps", bufs=2, space="PSUM")
            )
            stg = ph2.enter_context(tc.tile_pool(name="p2stg", bufs=4))
            smal = ph2.enter_context(tc.tile_pool(name="p2small", bufs=4))
            ostg01 = ph2.enter_context(tc.tile_pool(name="p2ostg01", bufs=2))

            o01_state = {"orow": None}

            def o01_groups():
                for mt in range(H // 128):
                    for ns in range(S // 512):
                        yield mt, ns

            o01_iter = o01_groups()

            def emit_o01_group():
                try:
                    mt, ns = next(o01_iter)
                except StopIteration:
                    return
                if ns == 0:
                    o01_state["orow"] = ostg01.tile(
                        [128, S], fmm, name=f"orow01_{mt}", tag="orow01"
                    )
                orow = o01_state["orow"]
                ps = o01_ps.tile([128, 512], f32, tag="o01ps")
                for a01 in (0, 1):
                    nc.tensor.matmul(
                        ps,
                        wo_sb[:, a01, 128 * mt : 128 * mt + 128],
                        attnT[:, a01, 512 * ns : 512 * ns + 512],
                        start=(a01 == 0),
                        stop=(a01 == 1),
                    )
                nc.vector.tensor_copy(orow[:, 512 * ns : 512 * (ns + 1)], ps)
                if ns == 3:
                    nc.gpsimd.dma_start(
                        out=outT2_d[128 * mt : 128 * mt + 128, :], in_=orow
                    )

            for a in range(NQ):
                kv = a // 2
                pT = ptp.tile([128, PT_TOTAL], fmm, tag="pT")
                # scores^T + exp, stripe per k-chunk c, only q >= 128c.
                # psum tiles are 1024 wide (2 banks): 2 matmuls fill, one
                # wide ScalarE exp drains (ACT paces this phase; fewer,
                # longer activations amortize its per-instruction cost).
                nfill = 0
                for c in range(ST):
                    off = STRIPE_OFF[c]
                    qlen = STRIPE_LEN[c]
                    lhsT = krot[:, kv, 128 * c : 128 * c + 128]
                    for sb in range((qlen + 1023) // 1024):
                        q0 = 128 * c + 1024 * sb
                        w = min(1024, S - q0)
                        ps = s_ps.tile([128, 1024], f32, tag="sps")
                        for h in range(0, w, 512):
                            hw = min(512, w - h)
                            nc.tensor.matmul(
                                ps[:, h : h + hw],
                                lhsT,
                                qrot[:, a, q0 + h : q0 + h + hw],
                                start=True,
                                stop=True,
                            )
                        nc.scalar.activation(
                            pT[:, off + q0 - 128 * c : off + q0 - 128 * c + w],
                            ps[:, :w],
                            mybir.ActivationFunctionType.Exp,
                            scale=float(SCALE),
                        )
                        nfill += 1
                        if a >= 2 and nfill > 2:
                            emit_o01_group()
                    # causal mask on the diagonal 128-block of this stripe
                    nc.vector.tensor_mul(
                        pT[:, off : off + 128], pT[:, off : off + 128], mask_sb
                    )
                # PV + normalize; the [q x d] -> [d x q] transpose runs
                # off-PE on the DMA XBAR via the sync ring.
                for t in range(ST):
                    po = pv_ps.tile([128, 132], f32, tag="pvps")
                    for c in range(t + 1):
                        lhsT = pT[
                            :,
                            STRIPE_OFF[c] + 128 * (t - c) : STRIPE_OFF[c]
                            + 128 * (t - c)
                            + 128,
                        ]
                        nc.tensor.matmul(
                            po[:, 0:129],
                            lhsT,
                            vaug[:, kv, c, 0:129],
                            start=(c == 0),
                            stop=(c == t),
                        )
                    r = smal.tile([128, 1], f32, tag="recip")
                    nc.vector.reciprocal(r, po[:, 128:129])
                    stage = stg.tile([128, 128], fmm, tag="stage")
                    nc.vector.tensor_scalar_mul(stage, po[:, 0:128], r)
                    nc.sync.dma_start(
                        out=attnT[:, a, 128 * t : 128 * t + 128],
                        in_=stage,
                        transpose=True,
                    )
                    if a >= 2:
                        emit_o01_group()

            # drain any heads-0/1 o_proj groups not yet emitted
            for _ in range(H // 128 * (S // 512)):
                emit_o01_group()

        # ---------------- phase 3: o_proj partial ----------------
        with ExitStack() as ph3:
            o_ps = ph3.enter_context(tc.tile_pool(name="p3ops", bufs=6, space="PSUM"))
            ostg = ph3.enter_context(tc.tile_pool(name="p3stg", bufs=3))

            for mt in range(H // 128):
                orow = ostg.tile([128, S], fmm, tag="ostg")
                for ns in range(S // 512):
                    ps = o_ps.tile([128, 512], f32, tag="ops")
                    for a in (2, 3):
                        nc.tensor.matmul(
                            ps,
                            wo_sb[:, a, 128 * mt : 128 * mt + 128],
                            attnT[:, a, 512 * ns : 512 * ns + 512],
                            start=(a == 2),
                            stop=(a == 3),
                        )
                    # alternate the psum drain between DVE and ACT so
                    # neither engine gates the o_proj matmul stream
                    if ns % 2 == 0:
                        nc.vector.tensor_copy(orow[:, 512 * ns : 512 * (ns + 1)], ps)
                    else:
                        nc.scalar.copy(orow[:, 512 * ns : 512 * (ns + 1)], ps)
                nc.sync.dma_start(
                    out=outT_d[128 * mt : 128 * mt + 128, :], in_=orow
                )

    nc.finalize()
    return nc


def _rope_tables():
    inv_freq = 1.0 / (10000.0 ** (np.arange(0, D, 2, dtype=np.float32) / D))
    t = np.arange(S, dtype=np.float32)[:, None]
    freqs = t * inv_freq[None, :]          # [S, 64]
    cos = np.cos(freqs).astype(np.float32)  # [S, 64]
    sin = np.sin(freqs).astype(np.float32)
    mdt = np.dtype(MM_DT)
    cosf = np.concatenate([cos, cos], axis=1).T.astype(mdt)    # [128, S]
    sins = np.concatenate([-sin, sin], axis=1).T.astype(mdt)   # [128, S]
    return np.ascontiguousarray(cosf), np.ascontiguousarray(sins)


def _prep_in_maps(hidden_states, Wq, Wk, Wv, Wo):
    mdt = np.dtype(MM_DT)
    cosf, sins = _rope_tables()
    mask = np.triu(np.ones((128, 128), dtype=mdt))  # [j, q]: 1 if j <= q

    hsT_blocks = []
    for b in range(B):
        hsT = hidden_states[b].T  # [H, S]
        blk = np.ascontiguousarray(
            hsT.reshape(HC, 128, NB, BW).transpose(2, 1, 0, 3).astype(mdt)
        )  # [NB, 128, HC, BW]
        hsT_blocks.append(blk)

    in_maps = []
    for i in range(8):
        b, g = i // 4, i % 4
        wq = np.ascontiguousarray(
            Wq[512 * g : 512 * (g + 1), :].reshape(512, HC, 128).transpose(2, 1, 0).astype(mdt)
        )
        wk = np.ascontiguousarray(
            Wk[256 * g : 256 * (g + 1), :].reshape(256, HC, 128).transpose(2, 1, 0).astype(mdt)
        )
        wv = np.ascontiguousarray(
            Wv[256 * g : 256 * (g + 1), :].reshape(256, HC, 128).transpose(2, 1, 0).astype(mdt)
        )
        wo = np.ascontiguousarray(
            Wo[:, 512 * g : 512 * (g + 1)].reshape(H, NQ, 128).transpose(2, 1, 0).astype(mdt)
        )
        in_maps.append(
            {
                "hsT": hsT_blocks[b],
                "wq": wq,
                "wk": wk,
                "wv": wv,
                "wo": wo,
                "cosf": cosf,
                "sins": sins,
                "mask": mask,
            }
        )
    return in_maps


def _run(in_maps, **kwargs):
    from concourse.bass_utils import run_bass_kernel_spmd

    if "prog" not in _CACHE:
        _CACHE["prog"] = _build_program()
    nc = _CACHE["prog"]
    return run_bass_kernel_spmd(nc, in_maps, core_ids=list(range(8)), **kwargs)


def _gather(results):
    out = np.empty((B, S, H), dtype=np.float32)
    for b in range(B):
        acc = results[4 * b + 0]["outT"].astype(np.float32)
        acc += results[4 * b + 0]["outT2"].astype(np.float32)
        for g in range(1, 4):
            acc += results[4 * b + g]["outT"].astype(np.float32)
            acc += results[4 * b + g]["outT2"].astype(np.float32)
        out[b] = acc.T
    return out


def kernel(hidden_states, Wq, Wk, Wv, Wo):
    hidden_states = np.asarray(hidden_states, dtype=np.float32)
    Wq = np.asarray(Wq, dtype=np.float32)
    Wk = np.asarray(Wk, dtype=np.float32)
    Wv = np.asarray(Wv, dtype=np.float32)
    Wo = np.asarray(Wo, dtype=np.float32)
    in_maps = _prep_in_maps(hidden_states, Wq, Wk, Wv, Wo)
    res = _run(in_maps)
    return _gather(res.results)


# revision 27
# speedup vs baseline: 1.1783x; 1.1783x over previous
"""Trainium2 Bass kernel for GQA attention block (nn_Attention_20272245637793).

Reference computation (B=2, S=2048, H=2048, 16 q heads / 8 kv heads, D=128):
    q = hs @ Wq.T ; k = hs @ Wk.T ; v = hs @ Wv.T
    rope(q), rope(k); causal softmax(q k^T / sqrt(D)) @ v ; out @ Wo.T

Sharding (8 cores): core i = (b, g) with b = i // 4 (data-parallel over
batch), g = i % 4 (tensor-parallel over kv-head groups; kv heads {2g, 2g+1},
q heads {4g..4g+3}).  Each core computes 1/8 of every GEMM and a partial
o_proj over its 512 head-dims; the host sums the 4 partials per batch
(cheap, off-device) instead of an on-device all-reduce.

Per-core dataflow (matmul operands fp16, PSUM accumulation fp32):
  phase 1: QK^T projections produce q^T/k^T in [d_head(part) x S(free)]
           layout directly (weights stationary, hs^T moving); RoPE applied
           on the PSUM->SBUF path with 4 DVE ops per tile using
           host-precomputed cos / (+/-)sin tables.  V is computed
           NON-transposed ([S x d]) by using hs^T slices as the stationary
           operand, and gets a ones-column appended (denominator trick).
  phase 2: per q head: scores^T tiles = K^T-chunk (stationary) @ q^T
           (moving) -> PSUM [k_pos(part) x q(free)] in 1024-wide 2-bank
           tiles (2 matmuls fill, ONE wide exp drains via ScalarE with
           scale=1/sqrt(D) fused); causal handled by skipping fully-masked
           tiles + one 0/1 mask multiply on diagonal tiles.  PV: exp'd
           score tiles are the stationary operand against V' (with ones
           column) -> PSUM [q(part) x 129]; col 128 is the softmax
           denominator; normalize with reciprocal + per-partition scalar
           multiply into an SBUF staging tile; the [q x d] -> [d x q]
           transpose for o_proj runs OFF the PE as a DMA XBAR transpose
           on the idle sync ring.
  phase 3: o_proj partial out^T[h, s] = Wo-slice^T (stationary) @ attn^T
           (moving), mt-outer so each 128-row strip is stored with ONE
           wide fp16 DMA; host sums/transposes partials in fp32.

DMA layout: all input tensors are shaped so per-partition lines are
4-16 KiB contiguous (whole-tensor or half-tensor DMAs) -- descriptor
count is what gates the kernel lead-in, not bandwidth.

Built on bacc.Bacc (not raw bass.Bass): TRN2 instructions can carry at most
ONE semaphore wait; Bacc.compile() legalizes multi-wait instructions via
move_matmul_waits_to_ldweights + generate_event_semaphores.
"""

import sys

sys.path.insert(0, "/opt/trn_rl_repo")

import numpy as np
from contextlib import ExitStack

B = 2
S = 2048
H = 2048
D = 128
NQ = 4          # q heads per core
NKVL = 2        # kv heads per core
HC = H // 128   # 16 h-chunks (contraction)
NB = 8          # hs^T column blocks of 256 for projections
BW = S // NB    # 256
ST = S // 128   # 16 s-tiles / k-chunks / q-tiles
SCALE = 1.0 / np.sqrt(D)

# stripe c of the exp'd transposed scores covers q in [128c, S); offsets of
# the stripes packed into one [128, sum] sbuf tile
STRIPE_LEN = [S - 128 * c for c in range(ST)]
STRIPE_OFF = np.concatenate([[0], np.cumsum(STRIPE_LEN)]).tolist()
PT_TOTAL = STRIPE_OFF[-1]  # 17408

# dtype for matmul operands (PSUM accumulation is always fp32).  fp32 runs
# every matmul as a hi/lo double pass on the PE; float16 is single-pass
# (2x) with ~1e-3 accuracy, and halves DMA/SBUF for those tensors.
MM_DT = "float16"

_CACHE = {}


def _build_program():
    import concourse.tile as tile
    from concourse import bacc, mybir

    f32 = mybir.dt.float32
    fmm = getattr(mybir.dt, MM_DT)
    nc = bacc.Bacc()

    hsT_d = nc.declare_dram_parameter("hsT", [NB, 128, HC, BW], fmm, isOutput=False)
    wq_d = nc.declare_dram_parameter("wq", [128, HC, 128 * NQ], fmm, isOutput=False)
    wk_d = nc.declare_dram_parameter("wk", [128, HC, 128 * NKVL], fmm, isOutput=False)
    wv_d = nc.declare_dram_parameter("wv", [128, HC, 128 * NKVL], fmm, isOutput=False)
    wo_d = nc.declare_dram_parameter("wo", [128, NQ, H], fmm, isOutput=False)
    cos_d = nc.declare_dram_parameter("cosf", [128, S], fmm, isOutput=False)
    sin_d = nc.declare_dram_parameter("sins", [128, S], fmm, isOutput=False)
    mask_d = nc.declare_dram_parameter("mask", [128, 128], fmm, isOutput=False)
    outT_d = nc.declare_dram_parameter("outT", [H, S], fmm, isOutput=True)

    with tile.TileContext(nc) as tc, ExitStack() as top:
        # tiles that live across phases
        glob = top.enter_context(tc.tile_pool(name="glob", bufs=1))
        qrot = glob.tile([128, NQ, S], fmm)      # q^T, rope'd, per head
        krot = glob.tile([128, NKVL, S], fmm)    # k^T, rope'd, per kv head
        vaug = glob.tile([128, NKVL, ST, 132], fmm)  # v chunks + ones col @128
        mask_sb = glob.tile([128, 128], fmm)

        nc.gpsimd.dma_start(out=mask_sb, in_=mask_d[:, :])
        nc.vector.memset(vaug[:, :, :, 128:129], 1.0)

        # HAM warm-up: ~10 throwaway matmuls on a zeroed scratch tile keep
        # the PE busy across the 3.4us activity window while the first
        # input DMAs land, so real matmuls start at the full clock.
        with ExitStack() as wctx:
            wpool = wctx.enter_context(tc.tile_pool(name="warm", bufs=1))
            wps_pool = wctx.enter_context(
                tc.tile_pool(name="warmps", bufs=1, space="PSUM")
            )
            warm = wpool.tile([128, 512], fmm)
            nc.vector.memset(warm, 0.0)
            wps = wps_pool.tile([128, 512], f32)
            for _ in range(10):
                nc.tensor.matmul(wps, warm[:, 0:128], warm, start=True, stop=True)

        # ---------------- phase 1: projections + rope ----------------
        with ExitStack() as ph1:
            consts = ph1.enter_context(tc.tile_pool(name="p1const", bufs=1))
            hsp = ph1.enter_context(tc.tile_pool(name="p1hs", bufs=3))
            ropep = ph1.enter_context(tc.tile_pool(name="p1rope", bufs=3))
            qk_ps = ph1.enter_context(tc.tile_pool(name="p1qkps", bufs=3, space="PSUM"))
            v_ps = ph1.enter_context(tc.tile_pool(name="p1vps", bufs=2, space="PSUM"))

            def hs_load(nb, splits=(4, 8, 12, 16)):
                # few DMAs per 1 MiB block with 2+ KiB contiguous
                # per-partition lines (vs 512 B per-chunk lines --
                # descriptor count gates the kernel lead-in, not bandwidth)
                t = hsp.tile([128, HC, BW], fmm, name=f"hs_{nb}", tag="hs")
                lo = 0
                for hi in splits:
                    nc.sync.dma_start(
                        out=t[:, lo:hi, :], in_=hsT_d[nb, :, lo:hi, :]
                    )
                    lo = hi
                return t

            wq_sb = consts.tile([128, HC, 128 * NQ], fmm)
            wk_sb = consts.tile([128, HC, 128 * NKVL], fmm)
            wv_sb = consts.tile([128, HC, 128 * NKVL], fmm)
            cos_sb = consts.tile([128, S], fmm)
            sin_sb = consts.tile([128, S], fmm)
            # scalar ring, in consumption order: a small first wq slice so
            # the very first matmuls can start, the rest of wq, then trig
            # in halves (rope of nb=0 only needs the first half).
            for lo, hi in ((0, 1), (1, 4), (4, 10), (10, 16)):
                nc.scalar.dma_start(
                    out=wq_sb[:, lo:hi, :], in_=wq_d[:, lo:hi, :]
                )
                if hi == 4:
                    nc.scalar.dma_start(
                        out=cos_sb[:, 0:1024], in_=cos_d[:, 0:1024]
                    )
                    nc.scalar.dma_start(
                        out=sin_sb[:, 0:1024], in_=sin_d[:, 0:1024]
                    )
            nc.scalar.dma_start(out=cos_sb[:, 1024:S], in_=cos_d[:, 1024:S])
            nc.scalar.dma_start(out=sin_sb[:, 1024:S], in_=sin_d[:, 1024:S])

            hs_next = hs_load(0, splits=(2, 4, 8, 12, 16))
            nc.sync.dma_start(out=wk_sb, in_=wk_d[:, :, :])
            nc.sync.dma_start(out=wv_sb, in_=wv_d[:, :, :])

            for nb in range(NB):
                n0 = nb * BW
                hs_t = hs_next
                if nb + 1 < NB:
                    hs_next = hs_load(nb + 1)

                # q/k projections (transposed out) + rope
                for mt in range(NQ + NKVL):
                    ps = qk_ps.tile([128, BW], f32)
                    if mt < NQ:
                        w_sb, mo = wq_sb, mt
                    else:
                        w_sb, mo = wk_sb, mt - NQ
                    for c in range(HC):
                        nc.tensor.matmul(
                            ps,
                            w_sb[:, c, 128 * mo : 128 * mo + 128],
                            hs_t[:, c, :],
                            start=(c == 0),
                            stop=(c == HC - 1),
                        )
                    if mt < NQ:
                        dest = qrot[:, mt, n0 : n0 + BW]
                    else:
                        dest = krot[:, mt - NQ, n0 : n0 + BW]
                    # rope: dest = ps * cos + swap_halves(ps) * (+/-)sin
                    t_t = ropep.tile([128, BW], f32, tag="ropet")
                    u_t = ropep.tile([128, BW], f32, tag="ropeu")
                    nc.vector.tensor_mul(t_t, ps, cos_sb[:, n0 : n0 + BW])
                    nc.vector.tensor_mul(
                        u_t[0:64, :], ps[64:128, :], sin_sb[0:64, n0 : n0 + BW]
                    )
                    nc.vector.tensor_mul(
                        u_t[64:128, :], ps[0:64, :], sin_sb[64:128, n0 : n0 + BW]
                    )
                    nc.vector.tensor_add(dest, t_t, u_t)

                # v projection (NOT transposed): out[s, d_local]
                for st2 in range(BW // 128):
                    st = (BW // 128) * nb + st2
                    ps = v_ps.tile([128, 128 * NKVL], f32)
                    for c in range(HC):
                        nc.tensor.matmul(
                            ps,
                            hs_t[:, c, 128 * st2 : 128 * st2 + 128],
                            wv_sb[:, c, :],
                            start=(c == 0),
                            stop=(c == HC - 1),
                        )
                    for kv in range(NKVL):
                        # ScalarE copy: ACT is idle during phase 1, DVE is not
                        nc.scalar.copy(
                            vaug[:, kv, st, 0:128], ps[:, 128 * kv : 128 * kv + 128]
                        )

        # ---------------- phases 2+3 ----------------
        late = top.enter_context(tc.tile_pool(name="late", bufs=1))
        attnT = late.tile([128, NQ, S], fmm)     # attention out, transposed
        wo_sb = late.tile([128, NQ, H], fmm)
        nc.gpsimd.dma_start(out=wo_sb, in_=wo_d[:, :, :])

        # ---------------- phase 2: attention ----------------
        with ExitStack() as ph2:
            ptp = ph2.enter_context(tc.tile_pool(name="p2pt", bufs=2))
            s_ps = ph2.enter_context(tc.tile_pool(name="p2sps", bufs=3, space="PSUM"))
            pv_ps = ph2.enter_context(tc.tile_pool(name="p2pvps", bufs=2, space="PSUM"))
            stg = ph2.enter_context(tc.tile_pool(name="p2stg", bufs=4))
            smal = ph2.enter_context(tc.tile_pool(name="p2small", bufs=4))

            for a in range(NQ):
                kv = a // 2
                pT = ptp.tile([128, PT_TOTAL], fmm, tag="pT")
                # scores^T + exp, stripe per k-chunk c, only q >= 128c.
                # psum tiles are 1024 wide (2 banks): 2 matmuls fill, one
                # wide ScalarE exp drains (ACT paces this phase; fewer,
                # longer activations amortize its per-instruction cost).
                for c in range(ST):
                    off = STRIPE_OFF[c]
                    qlen = STRIPE_LEN[c]
                    lhsT = krot[:, kv, 128 * c : 128 * c + 128]
                    for sb in range((qlen + 1023) // 1024):
                        q0 = 128 * c + 1024 * sb
                        w = min(1024, S - q0)
                        ps = s_ps.tile([128, 1024], f32, tag="sps")
                        for h in range(0, w, 512):
                            hw = min(512, w - h)
                            nc.tensor.matmul(
                                ps[:, h : h + hw],
                                lhsT,
                                qrot[:, a, q0 + h : q0 + h + hw],
                                start=True,
                                stop=True,
                            )
                        nc.scalar.activation(
                            pT[:, off + q0 - 128 * c : off + q0 - 128 * c + w],
                            ps[:, :w],
                            mybir.ActivationFunctionType.Exp,
                            scale=float(SCALE),
                        )
                    # causal mask on the diagonal 128-block of this stripe
                    nc.vector.tensor_mul(
                        pT[:, off : off + 128], pT[:, off : off + 128], mask_sb
                    )
                # PV + normalize; the [q x d] -> [d x q] transpose runs
                # off-PE on the DMA XBAR via the sync ring.
                for t in range(ST):
                    po = pv_ps.tile([128, 132], f32, tag="pvps")
                    for c in range(t + 1):
                        lhsT = pT[
                            :,
                            STRIPE_OFF[c] + 128 * (t - c) : STRIPE_OFF[c]
                            + 128 * (t - c)
                            + 128,
                        ]
                        nc.tensor.matmul(
                            po[:, 0:129],
                            lhsT,
                            vaug[:, kv, c, 0:129],
                            start=(c == 0),
                            stop=(c == t),
                        )
                    r = smal.tile([128, 1], f32, tag="recip")
                    nc.vector.reciprocal(r, po[:, 128:129])
                    stage = stg.tile([128, 128], fmm, tag="stage")
                    nc.vector.tensor_scalar_mul(stage, po[:, 0:128], r)
                    nc.sync.dma_start(
                        out=attnT[:, a, 128 * t : 128 * t + 128],
                        in_=stage,
                        transpose=True,
                    )

        # ---------------- phase 3: o_proj partial ----------------
        with ExitStack() as ph3:
            o_ps = ph3.enter_context(tc.tile_pool(name="p3ops", bufs=6, space="PSUM"))
            ostg = ph3.enter_context(tc.tile_pool(name="p3stg", bufs=3))

            for mt in range(H // 128):
                orow = ostg.tile([128, S], fmm, tag="ostg")
                for ns in range(S // 512):
                    ps = o_ps.tile([128, 512], f32, tag="ops")
                    for a in range(NQ):
                        nc.tensor.matmul(
                            ps,
                            wo_sb[:, a, 128 * mt : 128 * mt + 128],
                            attnT[:, a, 512 * ns : 512 * ns + 512],
                            start=(a == 0),
                            stop=(a == NQ - 1),
                        )
                    # alternate the psum drain between DVE and ACT so
                    # neither engine gates the o_proj matmul stream
                    if ns % 2 == 0:
                        nc.vector.tensor_copy(orow[:, 512 * ns : 512 * (ns + 1)], ps)
                    else:
                        nc.scalar.copy(orow[:, 512 * ns : 512 * (ns + 1)], ps)
                nc.sync.dma_start(
                    out=outT_d[128 * mt : 128 * mt + 128, :], in_=orow
                )

    nc.finalize()
    return nc


def _rope_tables():
    inv_freq = 1.0 / (10000.0 ** (np.arange(0, D, 2, dtype=np.float32) / D))
    t = np.arange(S, dtype=np.float32)[:, None]
    freqs = t * inv_freq[None, :]          # [S, 64]
    cos = np.cos(freqs).astype(np.float32)  # [S, 64]
    sin = np.sin(freqs).astype(np.float32)
    mdt = np.dtype(MM_DT)
    cosf = np.concatenate([cos, cos], axis=1).T.astype(mdt)    # [128, S]
    sins = np.concatenate([-sin, sin], axis=1).T.astype(mdt)   # [128, S]
    return np.ascontiguousarray(cosf), np.ascontiguousarray(sins)


def _prep_in_maps(hidden_states, Wq, Wk, Wv, Wo):
    mdt = np.dtype(MM_DT)
    cosf, sins = _rope_tables()
    mask = np.triu(np.ones((128, 128), dtype=mdt))  # [j, q]: 1 if j <= q

    hsT_blocks = []
    for b in range(B):
        hsT = hidden_states[b].T  # [H, S]
        blk = np.ascontiguousarray(
            hsT.reshape(HC, 128, NB, BW).transpose(2, 1, 0, 3).astype(mdt)
        )  # [NB, 128, HC, BW]
        hsT_blocks.append(blk)

    in_maps = []
    for i in range(8):
        b, g = i // 4, i % 4
        wq = np.ascontiguousarray(
            Wq[512 * g : 512 * (g + 1), :].reshape(512, HC, 128).transpose(2, 1, 0).astype(mdt)
        )
        wk = np.ascontiguousarray(
            Wk[256 * g : 256 * (g + 1), :].reshape(256, HC, 128).transpose(2, 1, 0).astype(mdt)
        )
        wv = np.ascontiguousarray(
            Wv[256 * g : 256 * (g + 1), :].reshape(256, HC, 128).transpose(2, 1, 0).astype(mdt)
        )
        wo = np.ascontiguousarray(
            Wo[:, 512 * g : 512 * (g + 1)].reshape(H, NQ, 128).transpose(2, 1, 0).astype(mdt)
        )
        in_maps.append(
            {
                "hsT": hsT_blocks[b],
                "wq": wq,
                "wk": wk,
                "wv": wv,
                "wo": wo,
                "cosf": cosf,
                "sins": sins,
                "mask": mask,
            }
        )
    return in_maps


def _run(in_maps, **kwargs):
    from concourse.bass_utils import run_bass_kernel_spmd

    if "prog" not in _CACHE:
        _CACHE["prog"] = _build_program()
    nc = _CACHE["prog"]
    return run_bass_kernel_spmd(nc, in_maps, core_ids=list(range(8)), **kwargs)


def _gather(results):
    out = np.empty((B, S, H), dtype=np.float32)
    for b in range(B):
        acc = results[4 * b + 0]["outT"].astype(np.float32)
        for g in range(1, 4):
            acc += results[4 * b + g]["outT"].astype(np.float32)
        out[b] = acc.T
    return out


def kernel(hidden_states, Wq, Wk, Wv, Wo):
    hidden_states = np.asarray(hidden_states, dtype=np.float32)
    Wq = np.asarray(Wq, dtype=np.float32)
    Wk = np.asarray(Wk, dtype=np.float32)
    Wv = np.asarray(Wv, dtype=np.float32)
    Wo = np.asarray(Wo, dtype=np.float32)
    in_maps = _prep_in_maps(hidden_states, Wq, Wk, Wv, Wo)
    res = _run(in_maps)
    return _gather(res.results)
